# revision 23
# baseline (speedup 1.0000x reference)
"""Single-head attention (B=8, S=2048, d_model=dk=dv=1024) on 8 TRN2 NeuronCores.

Strategy: data-parallel over batch — one batch element per core, SPMD.

Algebraic rewrite vs the direct form: scores = (xWq+bq)(xWk+bk)^T decomposes
into x·M·x^T + row-const + col-bias + const with M = Wq·Wk^T precomputed on
host. The row-constant and scalar terms are softmax-invariant and dropped;
the col-bias beta = x·(Wk bq) is folded into the exp's per-partition bias
operand. This needs ONE projection (t = x@M) on device instead of two (q, k).

Precision: the scores matmul t@x^T and the last 6/16 s-tiles of the
probs@V matmul run in fp8 e4m3 with perf_mode=DoubleRow (2 elements/
partition/cycle — measured 216ns per K=256,M=128,N=512 matmul, a full 2x
over bf16). The softmax's 1/sqrt(dk) scale damps the scores-fp8 noise
~30x; the PV fraction is sized so measured end-to-end rel err ~1.76e-2
stays under the 2e-2 gate. Everything else runs bf16 (fp16 measured 20%
slower per matmul on the moving-operand path; full-fp8 t/v/PV all fail
the gate).

Per-core phases:
  0.  ~192 tiny warm-up matmuls while the first DMAs land, so the PE_HAM
      clock gate (cold = 1.2 GHz) releases before the real work starts.
  1a. tT = M^T x^T (bf16 matmuls, fp32 PSUM) -> quantized straight to fp8
      in [128, 8, S] k-pair layout for DoubleRow.
  2.  scoresT[s',q] per 128-row block: 4 PSUM chains (one per 512-col q
      chunk) of 4 DoubleRow matmuls each; exp via scalar activation
      (scale=1/32, bias=scale*beta per partition). The last 6 s-blocks of
      probs are written straight to fp8 (they exist only as the PV
      DoubleRow operand).
  1b. v = x@Wv + bv (bf16; last 6 s-tiles written straight to fp8).
  3.  out = (probsT^T @ v) * (1/den): per qm-block, 10 bf16 s-tiles then
      3 fp8 DoubleRow s-pair matmuls; den[q] comes from an extra N=1
      matmul per (qm,sc) reusing the PV stationary against a ones vector,
      accumulated in a [P,1] PSUM column — this replaces the whole
      ones-stationary colsum pass (16us of PE) and the DRAM-bounce
      transpose of the denominator. Normalize alternates scalar/vector
      engines, writes fp16, host upcasts.
"""

import os
import sys

import numpy as np

try:
    import concourse.bass as bass  # noqa: F401
except ImportError:
    sys.path.insert(0, "/opt/trn_rl_repo")

import ml_dtypes

import concourse.bass as bass
import concourse.tile as tile
from concourse import bacc, mybir
from concourse import bass_utils

BF16 = mybir.dt.bfloat16
F16 = mybir.dt.float16
FP8 = mybir.dt.float8e4
F32 = mybir.dt.float32
DR = mybir.MatmulPerfMode.DoubleRow

B = 8
S = 2048
D = 1024  # d_model
DK = 1024
DV = 1024
P = 128  # partitions
NT = 512  # matmul free-dim tile (one PSUM bank of fp32)

D_T = D // P      # 8   contraction tiles over d_model
DK_T = DK // P    # 8   partition tiles of tT / fp8 k-chunks
C2 = DK_T // 2    # 4   DoubleRow k-pair count
S_T = S // P      # 16  partition tiles of v / probsT / out
S_N = S // NT     # 4   free-dim chunks over S
DV_N = DV // NT   # 2   free-dim chunks over dv

K8 = 6            # leading s-tiles of the PV matmul done in fp8 DoubleRow
N_WARM = 20       # HAM warm-up matmuls, N=512 for full array duty (N=1 MMs
                  # have ~3% duty and never trip the activity monitor)

SCALE = 1.0 / float(np.sqrt(np.float32(DK)))


def _emit(nc):
    xT_d = nc.dram_tensor("xT", [D, S], BF16, kind="ExternalInput").ap()
    x8_d = nc.dram_tensor("x8", [DK, S], FP8, kind="ExternalInput").ap()
    M_d = nc.dram_tensor("Mw", [D, DK], BF16, kind="ExternalInput").ap()
    Wv_d = nc.dram_tensor("Wv", [D, DV], BF16, kind="ExternalInput").ap()
    # bias pack: cols [0:S_T] = scale*beta per-partition (col sm for probsT
    # block sm), [S_T:S_T+DV] = bv replicated across partitions.
    bias_d = nc.dram_tensor("biases", [P, S_T + DV], F32, kind="ExternalInput").ap()
    out_d = nc.dram_tensor("out", [S, DV], F16, kind="ExternalOutput").ap()

    with tile.TileContext(nc) as tc:
        with tc.tile_pool(name="persist", bufs=1) as persist:
            x8 = persist.tile([P, DK_T * S], FP8, name="x8", tag="x8")
            t8 = persist.tile([P, DK_T * S], FP8, name="t8", tag="t8")
            v = [persist.tile([P, DV], BF16, name=f"v{i}", tag=f"v{i}")
                 for i in range(S_T - K8)]
            probs8 = persist.tile([P, K8 * S], FP8, name="probs8", tag="probs8")
            v8 = persist.tile([P, K8 * DV], FP8, name="vq8", tag="vq8")
            ones = persist.tile([P, 1], BF16, name="ones", tag="ones")
            ones8 = persist.tile([P, 2], FP8, name="ones8", tag="ones8")
            wmv = persist.tile([P, NT], BF16, name="wmv", tag="wmv")
            recip = persist.tile([P, S_T], F32, name="recip", tag="recip")
            bias = persist.tile([P, S_T + DV], F32, name="bias", tag="bias")
            nc.vector.memset(ones, 1.0)
            nc.vector.memset(ones8, 1.0)
            nc.vector.memset(wmv, 0.0)
            # pin lazily-allocated persist slots before the inp/probs pools
            # claim SBUF (slot acquisition is first-write ordered)
            nc.gpsimd.memset(probs8, 0.0)
            nc.gpsimd.memset(v8, 0.0)
            for vt in v:
                nc.gpsimd.memset(vt, 0.0)

            x8_3 = x8.rearrange("p (c s) -> p c s", c=DK_T)
            t8_3 = t8.rearrange("p (c s) -> p c s", c=DK_T)
            probs8_3 = probs8.rearrange("p (c s) -> p c s", c=K8)
            v8_3 = v8.rearrange("p (c s) -> p c s", c=K8)

            # Phase 0: warm the PE HAM clock gate while input DMAs land.
            # N=512 moving operand -> ~100% array duty -> un-throttle fires
            # ~3.4us in, before the real matmuls start.
            with tc.tile_pool(name="pwarm", bufs=1, space="PSUM") as pwarm:
                wps = pwarm.tile([1, NT], F32, name="wps", tag="wps")
                for _ in range(N_WARM):
                    nc.tensor.matmul(wps, ones, wmv, start=True, stop=True)

            with tc.tile_pool(name="inp", bufs=1) as inp:
                xTs = inp.tile([P, D_T * S], BF16, name="xTs", tag="xTs")
                Wvs = inp.tile([P, D_T * DV], BF16, name="Wvs", tag="Wvs")

                xT3 = xTs.rearrange("p (c s) -> p c s", c=D_T)
                Wv3 = Wvs.rearrange("p (c k) -> p c k", c=D_T)
                xTd3 = xT_d.rearrange("(c p) s -> p c s", p=P)
                Wvd3 = Wv_d.rearrange("(c p) k -> p c k", p=P)
                x8d3 = x8_d.rearrange("(c p) s -> p c s", p=P)

                with tc.tile_pool(name="mw", bufs=1) as mw:
                    Ms = mw.tile([P, D_T * DK], BF16, name="Ms", tag="Ms")
                    M3 = Ms.rearrange("p (c k) -> p c k", c=D_T)
                    Md3 = M_d.rearrange("(c p) k -> p c k", p=P)

                    # DMA order = consumption order; the first chain's
                    # operands are split across queues so the PE can start
                    # earlier than a single 1MB transfer would allow.
                    nc.sync.dma_start(out=M3[:, 0:4, 0:P], in_=Md3[:, 0:4, 0:P])
                    nc.sync.dma_start(out=M3[:, 4:8, 0:P], in_=Md3[:, 4:8, 0:P])
                    for kc2 in range(4):
                        nc.sync.dma_start(
                            out=xT3[:, 2 * kc2:2 * kc2 + 2, 0:NT],
                            in_=xTd3[:, 2 * kc2:2 * kc2 + 2, 0:NT],
                        )
                    for m in range(1, DK_T):
                        nc.sync.dma_start(
                            out=M3[:, :, m * P:(m + 1) * P],
                            in_=Md3[:, :, m * P:(m + 1) * P],
                        )
                    nc.sync.dma_start(out=bias, in_=bias_d)
                    for n in range(1, S_N):
                        for h in range(2):
                            nc.sync.dma_start(
                                out=xT3[:, 4 * h:4 * h + 4, n * NT:(n + 1) * NT],
                                in_=xTd3[:, 4 * h:4 * h + 4, n * NT:(n + 1) * NT],
                            )
                    nc.sync.dma_start(out=x8_3, in_=x8d3)
                    nc.sync.dma_start(out=Wvs, in_=Wvd3)

                    # Phase 1a: tT = M^T @ x^T, quantized to fp8 on copy-out.
                    with tc.tile_pool(name="ps1", bufs=8, space="PSUM") as ps1:
                        for n in range(S_N):
                            for m in range(DK_T):
                                ps = ps1.tile([P, NT], F32, name="ps_t", tag="ps1", bufs=8)
                                for kc in range(D_T):
                                    nc.tensor.matmul(
                                        ps,
                                        Ms[:, kc * DK + m * P: kc * DK + (m + 1) * P],
                                        xTs[:, kc * S + n * NT: kc * S + (n + 1) * NT],
                                        start=(kc == 0),
                                        stop=(kc == D_T - 1),
                                    )
                                nc.vector.tensor_copy(
                                    t8_3[:, m, n * NT:(n + 1) * NT], ps
                                )

                # Phase 2 (scores fp8 DoubleRow + exp), then 1b (v).
                with tc.tile_pool(name="probs", bufs=1) as probs_pool:
                    probsT = [
                        probs_pool.tile([P, S], BF16, name=f"pT{i}", tag=f"pT{i}")
                        for i in range(S_T - K8)
                    ]
                    with tc.tile_pool(name="ps2", bufs=8, space="PSUM") as ps2:
                        for sm in range(S_T):
                            pss = [
                                ps2.tile([P, NT], F32, name=f"ps_sc{n}", tag="ps2", bufs=8)
                                for n in range(S_N)
                            ]
                            for c2 in range(C2):
                                lhsT = x8_3[:, 2 * c2:2 * c2 + 2, sm * P:(sm + 1) * P]
                                for n in range(S_N):
                                    nc.tensor.matmul(
                                        pss[n],
                                        lhsT,
                                        t8_3[:, 2 * c2:2 * c2 + 2, n * NT:(n + 1) * NT],
                                        start=(c2 == 0),
                                        stop=(c2 == C2 - 1),
                                        perf_mode=DR,
                                    )
                            for n in range(S_N):
                                if sm < S_T - K8:
                                    dst = probsT[sm][:, n * NT:(n + 1) * NT]
                                else:
                                    # last K8 s-blocks live only in fp8 (PV
                                    # DoubleRow operand); exp writes it directly
                                    j = sm - (S_T - K8)
                                    dst = probs8[:, j * S + n * NT: j * S + (n + 1) * NT]
                                nc.scalar.activation(
                                    out=dst,
                                    in_=pss[n],
                                    func=mybir.ActivationFunctionType.Exp,
                                    scale=SCALE,
                                    bias=bias[:, sm:sm + 1],
                                )

                    # Phase 1b: v = x @ Wv + bv. Phase-3 PSUM pools open
                    # first so the pool turnover doesn't stall the PE.
                    with (
                        tc.tile_pool(name="ps3", bufs=2, space="PSUM") as ps3,
                        tc.tile_pool(name="psd", bufs=2, space="PSUM") as psd,
                        tc.tile_pool(name="ps1b", bufs=2, space="PSUM") as ps1b,
                    ):
                        for m in range(S_T):
                            for n in range(DV_N):
                                ps = ps1b.tile([P, NT], F32, name="ps_v", tag="ps1b", bufs=2)
                                for kc in range(D_T):
                                    nc.tensor.matmul(
                                        ps,
                                        xTs[:, kc * S + m * P: kc * S + (m + 1) * P],
                                        Wvs[:, kc * DV + n * NT: kc * DV + (n + 1) * NT],
                                        start=(kc == 0),
                                        stop=(kc == D_T - 1),
                                    )
                                if m < S_T - K8:
                                    vdst = v[m][:, n * NT:(n + 1) * NT]
                                else:
                                    j = m - (S_T - K8)
                                    vdst = v8[:, j * DV + n * NT: j * DV + (n + 1) * NT]
                                nc.vector.tensor_add(
                                    vdst,
                                    ps,
                                    bias[:, S_T + n * NT: S_T + (n + 1) * NT],
                                )

                        _phase3(nc, tc, ps3, psd, probsT, v, probs8_3, v8_3, ones, ones8, recip, out_d)


def _phase3(nc, tc, ps3, psd, probsT, v, probs8_3, v8_3, ones, ones8, recip, out_d):
    """out[qm*P+p, j] = (sum_s probsT[s, qm*P+p] * v[s, j]) / den[p, qm].
    The LAST K8 s-tiles of the numerator run as fp8 DoubleRow matmuls (the
    noise budget allows ~1/4 of the PV contraction in fp8; using the last
    tiles keeps the fp8 operands data-dependent on the whole v phase so the
    scheduler cannot hoist PV matmuls into a pool-boundary cycle). den[q]
    comes from an N=1 matmul per (qm, sc) sharing the PV stationary.
    Normalize alternates scalar/vector engines, writes fp16, host upcasts."""
    S_B = S_T - K8  # number of leading bf16 s-tiles
    with tc.tile_pool(name="outp", bufs=4) as outp:
        for qm in range(S_T):
            po = ps3.tile([P, DV], F32, name="po", tag="po", bufs=2)
            den = psd.tile([P, NT], F32, name="den", tag="den", bufs=2)
            for sc in range(S_B):
                lhsT = probsT[sc][:, qm * P:(qm + 1) * P]
                for nv in range(DV_N):
                    nc.tensor.matmul(
                        po[:, nv * NT:(nv + 1) * NT],
                        lhsT,
                        v[sc][:, nv * NT:(nv + 1) * NT],
                        start=(sc == 0),
                        stop=False,
                    )
                nc.tensor.matmul(den[:, 0:1], lhsT, ones, start=(sc == 0), stop=False)
            for c2p in range(K8 // 2):
                lhsT8 = probs8_3[:, 2 * c2p:2 * c2p + 2, qm * P:(qm + 1) * P]
                last = c2p == K8 // 2 - 1
                for nv in range(DV_N):
                    nc.tensor.matmul(
                        po[:, nv * NT:(nv + 1) * NT],
                        lhsT8,
                        v8_3[:, 2 * c2p:2 * c2p + 2, nv * NT:(nv + 1) * NT],
                        start=False,
                        stop=last,
                        perf_mode=DR,
                    )
                nc.tensor.matmul(
                    den[:, 0:1],
                    lhsT8,
                    ones8.rearrange("p (two o) -> p two o", two=2),
                    start=False,
                    stop=last,
                    perf_mode=DR,
                )
            nc.vector.reciprocal(recip[:, qm:qm + 1], den[:, 0:1])
            # last block: finer normalize/DMA chunks to shorten the kernel
            # tail (everything after the final matmul is pure latency)
            nch = 4 if qm == S_T - 1 else DV_N
            w = DV // nch
            for nv in range(nch):
                o = outp.tile([P, NT], F16, name="o", tag="o", bufs=4)
                if nv % 2 == 0:
                    nc.scalar.activation(
                        out=o[:, 0:w],
                        in_=po[:, nv * w:(nv + 1) * w],
                        func=mybir.ActivationFunctionType.Copy,
                        scale=recip[:, qm:qm + 1],
                    )
                else:
                    nc.vector.tensor_scalar_mul(
                        o[:, 0:w], po[:, nv * w:(nv + 1) * w], recip[:, qm:qm + 1]
                    )
                nc.sync.dma_start(
                    out=out_d[qm * P:(qm + 1) * P, nv * w:(nv + 1) * w],
                    in_=o[:, 0:w],
                )


_CACHED = None


def _build():
    global _CACHED
    if _CACHED is None:
        nc = bacc.Bacc(
            "TRN2",
            target_bir_lowering=False,
            debug=False,
            num_devices=B,
        )
        _emit(nc)
        nc.compile()
        _CACHED = nc
    return _CACHED


def _host_prep(x, Wq, bq, Wk, bk, Wv, bv):
    """Host-side preprocessing: M = Wq Wk^T, beta = x (Wk bq), layout packs."""
    bf = ml_dtypes.bfloat16
    f8 = ml_dtypes.float8_e4m3fn

    M64 = np.float64(Wq) @ np.float64(Wk).T
    M_h = np.ascontiguousarray(M64.astype(np.float32).astype(bf))
    Wv_h = np.ascontiguousarray(Wv.astype(bf))
    u = np.float64(Wk) @ np.float64(bq)          # [D]
    beta = np.float64(x) @ u                     # [B, S]

    in_maps = []
    for b in range(B):
        xb_T = np.ascontiguousarray(x[b].T)      # [D, S] f32
        bias_pack = np.empty((P, S_T + DV), dtype=np.float32)
        bias_pack[:, 0:S_T] = SCALE * beta[b].reshape(S_T, P).T
        bias_pack[:, S_T:] = bv[None, :]
        in_maps.append({
            "xT": xb_T.astype(bf),
            "x8": xb_T.astype(f8),
            "Mw": M_h,
            "Wv": Wv_h,
            "biases": bias_pack,
        })
    return in_maps


def kernel(x, Wq, bq, Wk, bk, Wv, bv):
    x = np.asarray(x, dtype=np.float32)
    Wq = np.asarray(Wq, dtype=np.float32)
    Wk = np.asarray(Wk, dtype=np.float32)
    Wv = np.asarray(Wv, dtype=np.float32)
    bq = np.asarray(bq, dtype=np.float32)
    bk = np.asarray(bk, dtype=np.float32)
    bv = np.asarray(bv, dtype=np.float32)

    in_maps = _host_prep(x, Wq, bq, Wk, bk, Wv, bv)

    nc = _build()
    res = bass_utils.run_bass_kernel_spmd(
        nc,
        in_maps,
        core_ids=list(range(B)),
        trace=bool(int(os.environ.get("KERNEL_TRACE", "0"))),
        tmpdir=os.environ.get("KERNEL_TRACE_DIR") or None,
    )
    kernel.last_result = res
    return np.stack([r["out"].astype(np.float32) for r in res.results], axis=0)
